# revision 25
# baseline (speedup 1.0000x reference)
"""ANI representation (radial + angular AEV) on 8 Trainium2 NeuronCores.

Strategy
--------
Data-parallel over atoms: each of the 8 cores owns 1024 atoms (after a
host-side similarity sort).  All floating-point math runs on-device; the
host only does index plumbing (building per-atom pair/triplet lists,
sort orders, gather indices, masks) and un-permutes the output.

Device algorithm per atom (one SBUF partition lane per atom):
  * slot stage: per incident pair ("slot"): unit vector u = r/|r|,
    |r| via exp/ln, angular cutoff term c1 = 1+cos(pi*min(d,A)/A) via an
    even polynomial, radial rbf = 0.25*exp(-eta_r (d - shfr)^2) * fc.
  * expansion: gpsimd indirect_copy gathers slot channels into the
    per-triplet (pair-of-pairs) layout.  Triplet enumeration is
    bucket-major over species-sorted padded slots, so each destination
    bucket is a contiguous segment.
  * triplet stage: cos_a via dot product, sin via exp(0.5*ln(1-c^2)),
    per-section cos(theta - phi_s) via Chebyshev recurrence,
    f1 = u^32 via exp(32*ln(u)) (single ACT table set: Ln+Exp),
    f2 gaussians, outer product -> ang[32] per triplet.
  * scatter-add: masked tensor_tensor_scan (state = m*state + ang)
    gives segmented sums; segment-end positions are gathered by a
    per-16-partition-group indirect_copy (positions are group data).
  * radial: same masked-scan trick over 16 rbf channels.

The per-atom data-dependence (species pattern) is absorbed into:
  (a) canonical per-atom species relabeling (descending count),
  (b) sorting atoms by relabeled count profile and padding slot
      segments to the max over each 16-atom group (~3% inflation),
  (c) per-group gather-index/mask tensors (DMA'd data),
  (d) a final host-side permutation of output rows/columns.
"""

import math
import os

import numpy as np

# ---- problem constants (from the reference nn.Module) ----
N_SPECIES = 7
N_PAIRS_SP = N_SPECIES * (N_SPECIES + 1) // 2  # 28
R_MAX, R_MIN, N_RBF, ETA_R = 0.51, 0.08, 16, 1600.0
A_MAX, A_MIN, N_DIV, N_SEC = 0.35, 0.08, 4, 8
ETA_A, ZETA = 800.0, 32.0
SHFR = np.linspace(R_MIN, R_MAX, N_RBF + 1)[:-1].astype(np.float64)
SHFA = np.linspace(A_MIN, A_MAX, N_DIV + 1)[:-1].astype(np.float64)
SHFZ = (np.linspace(0.0, np.pi, N_SEC + 1)[:-1] + np.pi / (2 * N_SEC)).astype(np.float64)

NCORES = 8
LANES = 128
GS = 16  # indirect_copy index-sharing group size
NFEAT = N_SEC * N_DIV  # 32
OUTW = N_SPECIES * N_RBF + N_PAIRS_SP * NFEAT  # 112 + 896 = 1008
NCH = 6  # slot channels: ux uy uz dist c1 sign

_SENT_R = (0.35, 0.0, 0.0)  # sentinel pair vector -> dist = A_MAX -> c1 ~ 0
_SENT_D = 0.51              # sentinel d for radial -> fc_R ~ 0


def _poly_coeffs(f, lo, hi, deg):
    """Least-squares/chebyshev polynomial coefficients (power basis)."""
    from numpy.polynomial import chebyshev as _C
    from numpy.polynomial import polynomial as _P
    ch = _C.Chebyshev.interpolate(f, deg, domain=[lo, hi])
    p = ch.convert(kind=np.polynomial.Polynomial)
    return np.asarray(p.coef, np.float64)


# c1(x) = 1 + cos(pi * min(x,A)/A) as even poly in t = x^2, t in [0, A^2]
_C1_COEF = _poly_coeffs(lambda t: 1.0 + np.cos(np.pi * np.sqrt(np.maximum(t, 0)) / A_MAX),
                        0.0, A_MAX * A_MAX, 10)
# cR1(x) = 1 + cos(pi * min(x,R)/R) in t = x^2, t in [0, R^2]
_CR1_COEF = _poly_coeffs(lambda t: 1.0 + np.cos(np.pi * np.sqrt(np.maximum(t, 0)) / R_MAX),
                         0.0, R_MAX * R_MAX, 10)


# ----------------------------------------------------------------------
# host-side index plumbing
# ----------------------------------------------------------------------

def _canonical_structure(n, p, species, atom_index12):
    """Incident pair list per atom, built directly from atom_index12.

    Returns pairs16 [N, 2K], sign16 [N, 2K] (+1: atom is first endpoint),
    osp16 [N, 2K] species of the other endpoint.  Requires every atom to
    have the same incidence count (true for the canonical half-list).
    """
    a0 = np.asarray(atom_index12[0], np.int64)
    a1 = np.asarray(atom_index12[1], np.int64)
    cnt = np.bincount(a0, minlength=n) + np.bincount(a1, minlength=n)
    assert cnt.min() == cnt.max(), "non-uniform incidence not supported"
    k2 = int(cnt[0])
    order0 = np.argsort(a0, kind="stable")
    order1 = np.argsort(a1, kind="stable")
    k = k2 // 2
    assert order0.shape[0] == n * k
    pairs16 = np.empty((n, k2), np.int64)
    sign16 = np.empty((n, k2), np.int8)
    pairs16[:, :k] = order0.reshape(n, k)
    pairs16[:, k:] = order1.reshape(n, k)
    sign16[:, :k] = 1
    sign16[:, k:] = -1
    other = np.where(sign16 > 0, a1[pairs16], a0[pairs16])
    osp16 = species[other]
    return pairs16, sign16, osp16


def _check_canonical_triplets(n, p, atom_index12, central, pair12, sign12):
    """Verify inputs match the canonical half-list ring construction, i.e.
    the triplet list is exactly all-pairs of each atom's incident pairs."""
    k = p // n
    a = np.repeat(np.arange(n), k)
    off = np.tile(np.arange(1, k + 1), n)
    if not (np.array_equal(atom_index12[0], a)
            and np.array_equal(atom_index12[1], (a + off) % n)):
        return False
    first = np.arange(p).reshape(n, k)
    ks = np.arange(1, k + 1)
    second = ((np.arange(n)[:, None] - ks[None, :]) % n) * k + (ks[None, :] - 1)
    inc = np.concatenate([first, second], axis=1)
    c0, c1 = np.triu_indices(2 * k, 1)
    m = c0.shape[0]
    if central.shape[0] != n * m:
        return False
    if not np.array_equal(central, np.repeat(np.arange(n), m)):
        return False
    if not (np.array_equal(pair12[0], inc[:, c0].reshape(-1))
            and np.array_equal(pair12[1], inc[:, c1].reshape(-1))):
        return False
    exp_sign = np.where(atom_index12[0][pair12] == central[None, :], 1, -1)
    if not np.array_equal(sign12, exp_sign.astype(sign12.dtype)):
        return False
    return True


def _group_consts(w, T, SS):
    """Per-group geometry from padded species widths w (len 7).

    Returns (mask[T], expidx[12T], angidx[896], radidx[112], mrad[SS])
    with slot index SS-1 reserved as the sentinel slot.
    """
    w = np.asarray(w, np.int64)
    offs = np.concatenate([[0], np.cumsum(w)])
    S = int(offs[-1])
    ncell = S * (S - 1) // 2
    assert ncell < T and S < SS
    jarr = np.full(T, SS - 1, np.int64)
    karr = np.full(T, SS - 1, np.int64)
    mask = np.ones(T, np.float32)
    Eb = np.full(N_PAIRS_SP, T - 1, np.int64)
    pos = 0
    b = 0
    for s1 in range(N_SPECIES):
        for s2 in range(s1, N_SPECIES):
            lo1, hi1 = offs[s1], offs[s1 + 1]
            lo2, hi2 = offs[s2], offs[s2 + 1]
            if s1 == s2:
                nc_ = (hi1 - lo1) * (hi1 - lo1 - 1) // 2
                if nc_ > 0:
                    jj, kk = np.triu_indices(hi1 - lo1, 1)
                    jarr[pos:pos + nc_] = lo1 + jj
                    karr[pos:pos + nc_] = lo1 + kk
            else:
                nc_ = (hi1 - lo1) * (hi2 - lo2)
                if nc_ > 0:
                    jj = np.repeat(np.arange(lo1, hi1), hi2 - lo2)
                    kk = np.tile(np.arange(lo2, hi2), hi1 - lo1)
                    jarr[pos:pos + nc_] = jj
                    karr[pos:pos + nc_] = kk
            if nc_ > 0:
                mask[pos] = 0.0
                pos += nc_
                Eb[b] = pos - 1
            b += 1
    assert pos == ncell
    mask[pos] = 0.0
    expidx = np.empty(12 * T, np.int64)
    for side, arr in ((0, jarr), (1, karr)):
        for ch in range(NCH):
            c = side * NCH + ch
            expidx[c * T:(c + 1) * T] = ch * SS + arr
    s_i = np.arange(N_SEC)
    d_i = np.arange(N_DIV)
    angidx = (s_i[None, :, None] * (N_DIV * T) + d_i[None, None, :] * T
              + Eb[:, None, None]).reshape(-1)
    Esp = np.full(N_SPECIES, SS - 1, np.int64)
    mrad = np.ones(SS, np.float32)
    for s in range(N_SPECIES):
        if w[s] > 0:
            mrad[offs[s]] = 0.0
            Esp[s] = offs[s + 1] - 1
    if S < SS:
        mrad[S] = 0.0
    radidx = (np.arange(N_RBF)[None, :] * SS + Esp[:, None]).reshape(-1)
    return mask, expidx, angidx, radidx, mrad


def _wrap16(lists, width, nlanes=LANES):
    """Wrap per-group index lists into the [128, W] layout indirect_copy
    expects: entry i of group g lives at [16*g + i%16, i//16]."""
    out = np.zeros((nlanes, width), np.uint16)
    for g, lst in enumerate(lists):
        arr = np.asarray(lst, np.int64)
        L = arr.shape[0]
        rows = GS * g + (np.arange(L) % GS)
        cols = np.arange(L) // GS
        out[rows, cols] = arr.astype(np.uint16)
    return out


def preprocess(r_ij, d_ij, species, atom_index12, central_atom_index,
               pair_index12, sign12, ncores=NCORES):
    """All host-side index plumbing.  Returns (shapes, in_maps, postinfo)."""
    r_ij = np.asarray(r_ij, np.float32)
    d_ij = np.asarray(d_ij, np.float32)
    species = np.asarray(species, np.int64)
    atom_index12 = np.asarray(atom_index12, np.int64)
    central = np.asarray(central_atom_index, np.int64)
    pair12 = np.asarray(pair_index12, np.int64)
    sign12 = np.asarray(sign12, np.int64)

    n = species.shape[0]
    p = r_ij.shape[0]
    pairs16, sign16, osp16 = _canonical_structure(n, p, species, atom_index12)
    k2 = pairs16.shape[1]
    assert _check_canonical_triplets(n, p, atom_index12, central, pair12,
                                     sign12), "non-canonical triplet list"

    # per-atom species relabel by descending count
    cnt_sp = np.zeros((n, N_SPECIES), np.int64)
    np.add.at(cnt_sp, (np.repeat(np.arange(n), k2), osp16.reshape(-1)), 1)
    perm = np.argsort(-cnt_sp, axis=1, kind="stable")       # rel -> orig species
    inv = np.argsort(perm, axis=1)                          # orig -> rel
    rel_cnt = np.take_along_axis(cnt_sp, perm, 1)           # descending

    # sort atoms by relabeled profile, group by 16
    atom_order = np.lexsort(tuple(rel_cnt[:, i] for i in range(N_SPECIES - 1, -1, -1)))
    ngroups = n // GS
    gcnt = rel_cnt[atom_order].reshape(ngroups, GS, N_SPECIES)
    w_all = gcnt.max(1)                                      # [G, 7]

    nlanes = LANES
    natoms_blk = nlanes
    nblocks = n // natoms_blk                   # global blocks of 128 sorted atoms
    nb_core = nblocks // ncores                 # blocks per core (= block slots)
    gpb = natoms_blk // GS                      # groups per block (8)

    # block-slot shapes shared across cores: slot j <- global blocks [8j, 8j+8)
    S_slot = np.empty(nb_core, np.int64)
    T_slot = np.empty(nb_core, np.int64)
    for j in range(nb_core):
        gsel = w_all[j * ncores * gpb:(j + 1) * ncores * gpb]
        Sg = gsel.sum(1)
        S_slot[j] = Sg.max()
        T_slot[j] = (Sg * (Sg - 1) // 2).max() + 1

    # relabeled species of each slot + species-sorted order per atom
    rel_osp = np.take_along_axis(inv, osp16, 1)              # [N, 16]
    slot_order = np.argsort(rel_osp, axis=1, kind="stable")  # sorted slot list

    shapes = []
    for j in range(nb_core):
        SS = int(S_slot[j]) + 1
        T = int(T_slot[j])
        assert 4 * T <= 1024, "indirect_copy index limit"
        WEg = ((4 * T + GS - 1) // GS + 1) // 2 * 2
        shapes.append((SS, T, 3 * WEg))

    in_maps = [dict() for _ in range(ncores)]
    const_cache = {}

    for j in range(nb_core):
        SS, T, WE = shapes[j]
        for c in range(ncores):
            gblk = j * ncores + c
            atoms = atom_order[gblk * natoms_blk:(gblk + 1) * natoms_blk]
            # ---- per-atom slot arrays ----
            rsl = np.empty((nlanes, 3, SS), np.float32)
            rsl[:, 0, :] = _SENT_R[0]
            rsl[:, 1, :] = _SENT_R[1]
            rsl[:, 2, :] = _SENT_R[2]
            dsl = np.full((nlanes, SS), _SENT_D, np.float32)
            ssl = np.ones((nlanes, SS), np.float32)

            so = slot_order[atoms]                            # [128, 16]
            rsp_sorted = np.take_along_axis(rel_osp[atoms], so, 1)
            # position: species-seg head + within-rank
            g_of_lane = gblk * gpb + np.arange(nlanes) // GS   # global group id
            wg = w_all[g_of_lane]                              # [128, 7]
            offs = np.concatenate([np.zeros((nlanes, 1), np.int64),
                                   np.cumsum(wg, 1)], 1)       # [128, 8]
            seg_start = np.take_along_axis(offs, rsp_sorted, 1)
            first_occ = np.zeros((nlanes, k2), np.int64)
            for s in range(N_SPECIES):
                is_s = rsp_sorted == s
                fo = np.where(is_s.any(1), is_s.argmax(1), 0)
                first_occ[is_s] = np.broadcast_to(fo[:, None], (nlanes, k2))[is_s]
            within = np.arange(k2)[None, :] - first_occ
            pos = seg_start + within                           # [128, 16]

            pair_sorted = np.take_along_axis(pairs16[atoms], so, 1)
            sign_sorted = np.take_along_axis(sign16[atoms], so, 1)
            lane_idx = np.repeat(np.arange(nlanes), k2)
            pos_f = pos.reshape(-1)
            pr = pair_sorted.reshape(-1)
            rsl[lane_idx, 0, pos_f] = r_ij[pr, 0]
            rsl[lane_idx, 1, pos_f] = r_ij[pr, 1]
            rsl[lane_idx, 2, pos_f] = r_ij[pr, 2]
            dsl[lane_idx, pos_f] = d_ij[pr, 0]
            ssl[lane_idx, pos_f] = sign_sorted.reshape(-1).astype(np.float32)

            # ---- per-group geometry ----
            masks, expl, angl, radl, mradl = [], [], [], [], []
            for g in range(gpb):
                key = (tuple(w_all[gblk * gpb + g]), T, SS)
                if key not in const_cache:
                    const_cache[key] = _group_consts(key[0], T, SS)
                mk, ei, ai, ri, mr = const_cache[key]
                masks.append(mk)
                expl.append(ei)
                angl.append(ai)
                radl.append(ri)
                mradl.append(mr)
            mang = np.repeat(np.stack(masks), GS, axis=0)      # [128, T]
            mrad = np.repeat(np.stack(mradl), GS, axis=0)      # [128, SS]
            WEg = WE // 3
            expidx = np.concatenate(
                [_wrap16([e[g * 4 * T:(g + 1) * 4 * T] for e in expl], WEg)
                 for g in range(3)], axis=1)
            angidx = _wrap16(angl, (N_PAIRS_SP * NFEAT + GS - 1) // GS)
            radidx = _wrap16(radl, (N_SPECIES * N_RBF + GS - 1) // GS)

            blob = np.concatenate([
                rsl.reshape(nlanes, 3 * SS).view(np.uint8),
                dsl.view(np.uint8),
                ssl.view(np.uint8),
                mang.view(np.uint8),
                mrad.view(np.uint8),
                expidx.view(np.uint8),
                angidx.view(np.uint8),
                radidx.view(np.uint8),
            ], axis=1)
            if blob.shape[1] % 4:
                blob = np.concatenate(
                    [blob, np.zeros((nlanes, 4 - blob.shape[1] % 4), np.uint8)], 1)
            in_maps[c][f"blob{j}"] = np.ascontiguousarray(blob)

    # ---- output permutation info ----
    sortpos = np.argsort(atom_order)            # orig atom -> sorted index
    # sorted index m -> device row: core = (m//128) % ncores, j = m//(128*ncores)
    mm = np.arange(n)
    core_of = (mm // natoms_blk) % ncores
    j_of = mm // (natoms_blk * ncores)
    devrow = core_of * (nb_core * natoms_blk) + j_of * natoms_blk + (mm % natoms_blk)
    row_of_atom = devrow[sortpos]               # orig atom -> concat row

    # column map: out col -> device col, per atom
    triu_dev = np.zeros((N_SPECIES, N_SPECIES), np.int64)
    b = 0
    for s1 in range(N_SPECIES):
        for s2 in range(s1, N_SPECIES):
            triu_dev[s1, s2] = triu_dev[s2, s1] = b
            b += 1
    # reference TRIU (same construction)
    triu_ref = triu_dev
    colmap = np.empty((n, OUTW), np.int64)
    r_i = np.arange(N_RBF)
    f_i = np.arange(NFEAT)
    for a_ in range(0, n, 1024):
        sl = slice(a_, a_ + 1024)
        inv_sl = inv[sl]                                     # [1024, 7]
        # radial: out col sp_o*16 + r <- dev col rel*16 + r
        rel = inv_sl                                          # orig sp -> rel
        colmap[sl, :N_SPECIES * N_RBF] = (
            rel[:, :, None] * N_RBF + r_i[None, None, :]).reshape(-1, N_SPECIES * N_RBF)
        # angular: out col triu_ref(s1o,s2o)*32+f <- dev triu_dev(rel1,rel2)*32+f
        s1o, s2o = np.triu_indices(N_SPECIES)
        rel1 = inv_sl[:, s1o]
        rel2 = inv_sl[:, s2o]
        bdev = triu_dev[rel1, rel2]                           # [1024, 28]
        bref = triu_ref[s1o, s2o]                             # [28]
        blockcols = np.empty((inv_sl.shape[0], N_PAIRS_SP), np.int64)
        blockcols[:, bref] = bdev
        colmap[sl, N_SPECIES * N_RBF:] = (
            N_SPECIES * N_RBF + blockcols[:, :, None] * NFEAT
            + f_i[None, None, :]).reshape(-1, N_PAIRS_SP * NFEAT)

    return shapes, in_maps, (row_of_atom, colmap)


# ----------------------------------------------------------------------
# device kernel builder
# ----------------------------------------------------------------------

def build_kernel(shapes, nb_core):
    import concourse.bass as bass
    import concourse.mybir as mybir
    from concourse.bacc import Bacc
    from concourse.tile import TileContext

    f32 = mybir.dt.float32
    u16 = mybir.dt.uint16
    Alu = mybir.AluOpType
    Act = mybir.ActivationFunctionType

    nc = Bacc(monotonic_sem_count=0)
    u8 = mybir.dt.uint8
    prm = {}
    blob_bytes = []
    for j, (SS, T, WE) in enumerate(shapes):
        WA = (N_PAIRS_SP * NFEAT + GS - 1) // GS
        WR = (N_SPECIES * N_RBF + GS - 1) // GS
        nb = 4 * (3 * SS + SS + SS + T + SS) + 2 * (WE + WA + WR)
        nb = (nb + 3) // 4 * 4
        blob_bytes.append(nb)
        prm[f"blob{j}"] = nc.declare_dram_parameter(f"blob{j}", [LANES, nb], u8, False)
    out_ext = nc.declare_dram_parameter("out", [nb_core * LANES, OUTW], f32, True)

    V = nc.vector
    A = nc.scalar
    G = nc.gpsimd

    # activation scale/bias floats need registered const APs
    def _reg_const(*vals):
        for v in vals:
            v = float(v)
            if (f32, v) in nc.const_aps.aps:
                continue
            t_ = nc.alloc_sbuf_tensor(f"constf32_{len(nc.const_aps.aps)}", [128, 1], f32)
            nc.gpsimd.memset(t_.ap(), v)
            nc.const_aps.aps[(f32, v)] = t_.ap()

    _reg_const(-0.5, 0.5, 0.5 + 1e-30, float(ZETA), -ETA_R, -ETA_A)
    nc.all_engine_barrier()

    # Only Ln/Exp are used; force the single table set containing both so
    # Bacc inserts exactly one LoadActFuncSet instead of thrashing tables.
    import types

    import bass_rust as _br
    from concourse.hw_specs import get_activation_tables

    _tables = get_activation_tables(nc.m.arch)
    _keep = "natural_log_exp_and_others"
    if _keep in _tables:
        _filt = [(k, (v if k == _keep else set())) for k, v in _tables.items()]

        def _patched_insert(self, _filt=_filt):
            has_activation = any(
                isinstance(i, mybir.InstActivation)
                for b in self.main_func.blocks
                for i in b.instructions
            )
            if has_activation:
                _br.insert_act_table_loads(self, _filt)

        nc.insert_act_table_loads = types.MethodType(_patched_insert, nc)

    a_cheb = np.cos(SHFZ)          # cos(phi_s)
    b_cheb = np.sin(SHFZ)
    two_cos_d = 2.0 * math.cos(math.pi / N_SEC)

    with TileContext(nc) as tc:
        with tc.tile_pool(name="pool", bufs=2) as pool:
            for j, (SS, T, WE) in enumerate(shapes):
                # ---------- DMA inputs (one packed blob per block) ----------
                WA = (N_PAIRS_SP * NFEAT + GS - 1) // GS
                WR = (N_SPECIES * N_RBF + GS - 1) // GS
                blob = pool.tile([LANES, blob_bytes[j]], u8, tag=f"blob{j}", bufs=1)
                G.dma_start(out=blob[:, :], in_=prm[f"blob{j}"][:, :])
                off = 0

                def _sec(nelem, dt_, esz):
                    nonlocal off
                    v = blob[:, off:off + nelem * esz].bitcast(dt_)
                    off += nelem * esz
                    return v

                rsl = _sec(3 * SS, f32, 4)
                dsl = _sec(SS, f32, 4)
                ssl = _sec(SS, f32, 4)
                mang = _sec(T, f32, 4)
                mrad = _sec(SS, f32, 4)
                idx_view = _sec(WE + WA + WR, u16, 2)
                # copy indices onto the Pool engine so the 1-wait-slot
                # gpsimd ISA ops depend on them via program order only
                idxs = pool.tile([LANES, WE + WA + WR], u16, tag=f"idxs{j}", bufs=1)
                V.tensor_copy(idxs[:, :], idx_view)
                expidx = idxs[:, 0:WE]
                angidx = idxs[:, WE:WE + WA]
                radidx = idxs[:, WE + WA:WE + WA + WR]

                # ---------- slot stage ----------
                slotch = pool.tile([LANES, NCH * SS], f32, tag="slotch")
                tmpS = pool.tile([LANES, 3 * SS], f32, tag="tmpS")
                # sumsq
                V.tensor_tensor(tmpS[:, :], rsl[:, :], rsl[:, :], Alu.mult)
                sumsq = pool.tile([LANES, SS], f32, tag="sumsq")
                V.tensor_tensor(sumsq[:, :], tmpS[:, 0:SS], tmpS[:, SS:2 * SS], Alu.add)
                V.tensor_tensor(sumsq[:, :], sumsq[:, :], tmpS[:, 2 * SS:3 * SS], Alu.add)
                lnss = pool.tile([LANES, SS], f32, tag="lnss")
                A.activation(lnss[:, :], sumsq[:, :], Act.Ln)
                invd = pool.tile([LANES, SS], f32, tag="invd")
                A.activation(invd[:, :], lnss[:, :], Act.Exp, scale=-0.5)
                # dist = sumsq * invd -> channel 3
                V.tensor_tensor(slotch[:, 3 * SS:4 * SS], sumsq[:, :], invd[:, :], Alu.mult)
                # u = r * invd -> channels 0..2
                rv = rsl[:, :].rearrange("p (c s) -> p c s", c=3)
                iv = invd[:, :].unsqueeze(1).broadcast_to([LANES, 3, SS])
                uv = slotch[:, 0:3 * SS].rearrange("p (c s) -> p c s", c=3)
                V.tensor_tensor(uv, rv, iv, Alu.mult)
                # c1 = poly(min(dist,A)^2) -> channel 4
                tA = pool.tile([LANES, SS], f32, tag="tA")
                V.tensor_scalar_min(tA[:, :], slotch[:, 3 * SS:4 * SS], A_MAX)
                V.tensor_tensor(tA[:, :], tA[:, :], tA[:, :], Alu.mult)
                h = slotch[:, 4 * SS:5 * SS]
                V.memset(h, float(_C1_COEF[-1]))
                for ci in range(len(_C1_COEF) - 2, -1, -1):
                    V.tensor_tensor(h, h, tA[:, :], Alu.mult)
                    V.tensor_scalar_add(h, h, float(_C1_COEF[ci]))
                # sign channel 5
                V.tensor_copy(slotch[:, 5 * SS:6 * SS], ssl[:, :])

                # ---------- radial ----------
                tR = pool.tile([LANES, SS], f32, tag="tR")
                V.tensor_scalar_min(tR[:, :], dsl[:, :], R_MAX)
                V.tensor_tensor(tR[:, :], tR[:, :], tR[:, :], Alu.mult)
                cR1 = pool.tile([LANES, SS], f32, tag="cR1")
                V.memset(cR1[:, :], float(_CR1_COEF[-1]))
                for ci in range(len(_CR1_COEF) - 2, -1, -1):
                    V.tensor_tensor(cR1[:, :], cR1[:, :], tR[:, :], Alu.mult)
                    V.tensor_scalar_add(cR1[:, :], cR1[:, :], float(_CR1_COEF[ci]))
                rfv = pool.tile([LANES, N_RBF * SS], f32, tag="rfv")
                rfv3 = rfv[:, :].rearrange("p (r s) -> p r s", r=N_RBF)
                dvb = dsl[:, :].unsqueeze(1).broadcast_to([LANES, N_RBF, SS])
                # diff = d - shfr  (per-r scalar via separate slices)
                for r_ in range(N_RBF):
                    V.tensor_scalar_add(rfv[:, r_ * SS:(r_ + 1) * SS], dsl[:, :],
                                        float(-SHFR[r_]))
                V.tensor_tensor(rfv[:, :], rfv[:, :], rfv[:, :], Alu.mult)
                A.activation(rfv[:, :], rfv[:, :], Act.Exp, scale=-ETA_R)
                crb = cR1[:, :].unsqueeze(1).broadcast_to([LANES, N_RBF, SS])
                V.scalar_tensor_tensor(rfv3, rfv3, 0.125, crb, Alu.mult, Alu.mult)
                # masked scan over slots (chained across r)
                mr16 = pool.tile([LANES, N_RBF * SS], f32, tag="mr16")
                mrb = mrad[:, :].unsqueeze(1).broadcast_to([LANES, N_RBF, SS])
                V.tensor_copy(mr16[:, :].rearrange("p (r s) -> p r s", r=N_RBF), mrb)
                V.tensor_tensor_scan(rfv[:, :], mr16[:, :], rfv[:, :], 0.0,
                                     Alu.mult, Alu.add)

                staging = pool.tile([LANES, OUTW], f32, tag=f"staging{j}", bufs=1)
                # Pool fences: a TPB-class Pool op reading the gather inputs
                # advances Pool's observed DVE clock so the 1-wait-slot
                # gpsimd ISA gathers below need no cross-engine waits.
                scr = pool.tile([LANES, 32], f32, tag=f"scr{j}", bufs=1)
                scr16 = pool.tile([LANES, 4], u16, tag=f"scr16{j}", bufs=1)
                G.tensor_copy(scr16[:, 0:2], idxs[:, 0:2])
                G.tensor_copy(scr[:, 2:4], rfv[:, 0:2])
                G.indirect_copy(staging[:, 0:N_SPECIES * N_RBF], rfv[:, :],
                                radidx, True)

                STAGE = int(os.environ.get("ANI_STAGE", "5"))
                if STAGE < 5:
                    G.memset(staging[:, N_SPECIES * N_RBF:OUTW], 0.0)
                if STAGE < 2:
                    nc.sync.dma_start(out=out_ext[j * LANES:(j + 1) * LANES, :],
                                      in_=staging[:, :])
                    continue
                # ---------- expansion ----------
                exp_ = pool.tile([LANES, 12 * T], f32, tag="exp")
                slot_span = slotch[:, :].rearrange("p (c s) -> p c s", c=NCH)[:, :, 0:2]
                G.tensor_copy(scr[:, 4:4 + 2 * NCH], slot_span)
                WEg = WE // 3
                for g_ in range(3):
                    G.indirect_copy(exp_[:, g_ * 4 * T:(g_ + 1) * 4 * T],
                                    slotch[:, :],
                                    expidx[:, g_ * WEg:(g_ + 1) * WEg], True)
                if STAGE < 3:
                    nc.sync.dma_start(out=out_ext[j * LANES:(j + 1) * LANES, :],
                                      in_=staging[:, :])
                    continue
                ujx = exp_[:, 0 * T:3 * T]
                dj = exp_[:, 3 * T:4 * T]
                c1j = exp_[:, 4 * T:5 * T]
                sj = exp_[:, 5 * T:6 * T]
                ukx = exp_[:, 6 * T:9 * T]
                dk = exp_[:, 9 * T:10 * T]
                c1k = exp_[:, 10 * T:11 * T]
                sk = exp_[:, 11 * T:12 * T]

                # ---------- triplet stage ----------
                prods = pool.tile([LANES, 3 * T], f32, tag="prods")
                V.tensor_tensor(prods[:, :], ujx, ukx, Alu.mult)
                dot = pool.tile([LANES, T], f32, tag="dot")
                V.tensor_tensor(dot[:, :], prods[:, 0:T], prods[:, T:2 * T], Alu.add)
                V.tensor_tensor(dot[:, :], dot[:, :], prods[:, 2 * T:3 * T], Alu.add)
                cosa = pool.tile([LANES, T], f32, tag="cosa")
                V.scalar_tensor_tensor(cosa[:, :], dot[:, :], 0.95, sj, Alu.mult, Alu.mult)
                V.tensor_tensor(cosa[:, :], cosa[:, :], sk, Alu.mult)
                # sin = exp(0.5*ln(1-cos^2))
                nc2 = pool.tile([LANES, T], f32, tag="nc2")
                V.scalar_tensor_tensor(nc2[:, :], cosa[:, :], -1.0, cosa[:, :],
                                       Alu.mult, Alu.mult)
                A.activation(nc2[:, :], nc2[:, :], Act.Ln, bias=1.0)
                sina = pool.tile([LANES, T], f32, tag="sina")
                A.activation(sina[:, :], nc2[:, :], Act.Exp, scale=0.5)

                # chebyshev recurrence for cos(theta - phi_s)
                cbuf = pool.tile([LANES, N_SEC * T], f32, tag="cbuf")
                c0 = cbuf[:, 0:T]
                V.scalar_tensor_tensor(c0, sina[:, :], float(b_cheb[0] / a_cheb[0]),
                                       cosa[:, :], Alu.mult, Alu.add)
                V.tensor_scalar_mul(c0, c0, float(a_cheb[0]))
                c1_ = cbuf[:, T:2 * T]
                V.scalar_tensor_tensor(c1_, sina[:, :], float(b_cheb[1] / a_cheb[1]),
                                       cosa[:, :], Alu.mult, Alu.add)
                V.tensor_scalar_mul(c1_, c1_, float(a_cheb[1]))
                for s_ in range(2, N_SEC):
                    V.scalar_tensor_tensor(cbuf[:, s_ * T:(s_ + 1) * T],
                                           cbuf[:, (s_ - 1) * T:s_ * T], two_cos_d,
                                           cbuf[:, (s_ - 2) * T:(s_ - 1) * T],
                                           Alu.mult, Alu.subtract)
                # f1 = exp(32*ln(0.5 + 0.5*c))
                f1 = pool.tile([LANES, N_SEC * T], f32, tag="f1")
                A.activation(f1[:, :], cbuf[:, :], Act.Ln, bias=0.5 + 1e-30, scale=0.5)
                A.activation(f1[:, :], f1[:, :], Act.Exp, scale=ZETA)

                # f2 part
                dsum = pool.tile([LANES, T], f32, tag="dsum")
                V.tensor_tensor(dsum[:, :], dj, dk, Alu.add)
                f2 = pool.tile([LANES, N_DIV * T], f32, tag="f2")
                for d_ in range(N_DIV):
                    V.tensor_scalar(f2[:, d_ * T:(d_ + 1) * T], dsum[:, :], 0.5,
                                    float(-SHFA[d_]), Alu.mult, Alu.add)
                V.tensor_tensor(f2[:, :], f2[:, :], f2[:, :], Alu.mult)
                A.activation(f2[:, :], f2[:, :], Act.Exp, scale=-ETA_A)
                w2 = pool.tile([LANES, T], f32, tag="w2")
                V.scalar_tensor_tensor(w2[:, :], c1j, 0.5, c1k, Alu.mult, Alu.mult)
                w2b = w2[:, :].unsqueeze(1).broadcast_to([LANES, N_DIV, T])
                f23 = f2[:, :].rearrange("p (d t) -> p d t", d=N_DIV)
                V.tensor_tensor(f23, f23, w2b, Alu.mult)

                if STAGE < 4:
                    nc.sync.dma_start(out=out_ext[j * LANES:(j + 1) * LANES, :],
                                      in_=staging[:, :])
                    continue
                # ang = f1 (x) f2
                ang = pool.tile([LANES, NFEAT * T], f32, tag="ang")
                ang4 = ang[:, :].rearrange("p (s d t) -> p s d t", s=N_SEC, d=N_DIV)
                f13 = f1[:, :].rearrange("p (s t) -> p s t", s=N_SEC)
                for d_ in range(N_DIV):
                    f2b = f2[:, d_ * T:(d_ + 1) * T].unsqueeze(1).broadcast_to(
                        [LANES, N_SEC, T])
                    V.tensor_tensor(ang4[:, :, d_, :], f13, f2b, Alu.mult)

                # masked scan per section (chained across d)
                mask4 = pool.tile([LANES, N_DIV * T], f32, tag="mask4")
                mb = mang[:, :].unsqueeze(1).broadcast_to([LANES, N_DIV, T])
                V.tensor_copy(mask4[:, :].rearrange("p (d t) -> p d t", d=N_DIV), mb)
                for s_ in range(N_SEC):
                    seg = ang[:, s_ * N_DIV * T:(s_ + 1) * N_DIV * T]
                    V.tensor_tensor_scan(seg, mask4[:, :], seg, 0.0, Alu.mult, Alu.add)

                if STAGE < 5:
                    nc.sync.dma_start(out=out_ext[j * LANES:(j + 1) * LANES, :],
                                      in_=staging[:, :])
                    continue
                # extraction
                ang_span = ang[:, :].rearrange("p (s x) -> p s x", s=N_SEC)[:, :, 0:2]
                G.tensor_copy(scr[:, 16:16 + 2 * N_SEC], ang_span)
                G.indirect_copy(staging[:, N_SPECIES * N_RBF:OUTW], ang[:, :],
                                angidx, True)

                # output
                nc.sync.dma_start(out=out_ext[j * LANES:(j + 1) * LANES, :],
                                  in_=staging[:, :])
    return nc


def _strip_dead_reg_insts(nc):
    """Remove dead engine-preamble register defs (RegisterMove/tpb_base_ld)
    that Bacc's dce misses; walrus aborts on unallocated register defs."""
    from collections import Counter

    def regrefs(args):
        out = []
        for a in args:
            r = getattr(a, "regref", None)
            if r is not None:
                out.append(r)
        return out

    funcs = nc.m.functions
    uses = Counter()
    for f in funcs:
        for b in f.blocks:
            for i in b.instructions:
                for r in regrefs(list(i.ins) + list(i.outs)):
                    uses[r] += 1

    def dead(i):
        outs = list(i.outs)
        if not outs:
            return False
        rr = regrefs(outs)
        if len(rr) != len(outs):
            return False
        if regrefs(list(i.ins)):
            return False
        for a in i.ins:
            if getattr(a, "regref", None) is None and not hasattr(a, "value"):
                return False
        return all(uses[r] == sum(1 for x in rr if x == r) for r in rr)

    removed = 0
    for f in funcs:
        for b in f.blocks:
            keep = []
            for i in b.instructions:
                if dead(i):
                    removed += 1
                else:
                    keep.append(i)
            b.instructions = keep
    return removed


# ----------------------------------------------------------------------
# NTFF profiling shim (the container's antenv lacks axon_hooks)
# ----------------------------------------------------------------------

def _install_trace_hook(so_path="/opt/axon/libaxon_pjrt.so"):
    import contextlib
    import ctypes
    import sys
    import types

    try:
        import antenv.axon_hooks  # noqa: F401
        return
    except ImportError:
        pass
    if not os.path.exists(so_path):
        return
    lib = ctypes.CDLL(so_path)
    if not hasattr(lib, "axon_start_nrt_profile"):
        return
    lib.axon_start_nrt_profile.argtypes = [ctypes.POINTER(ctypes.c_int64),
                                           ctypes.c_size_t]
    lib.axon_start_nrt_profile.restype = ctypes.c_int64
    lib.axon_stop_nrt_profile.argtypes = [ctypes.c_char_p]
    lib.axon_stop_nrt_profile.restype = ctypes.c_int64

    @contextlib.contextmanager
    def _hook(output_dir, device_ids):
        import jax
        jax.devices()
        if device_ids:
            ids = (ctypes.c_int64 * len(device_ids))(*device_ids)
            rc = lib.axon_start_nrt_profile(ids, len(device_ids))
        else:
            rc = lib.axon_start_nrt_profile(None, 0)
        if rc != 0:
            raise RuntimeError(f"axon_start_nrt_profile rc={rc}")
        try:
            yield
        finally:
            n = lib.axon_stop_nrt_profile(str(output_dir).encode())
            print(f"ntff profile: {n} file(s) -> {output_dir}")

    mod = types.ModuleType("antenv.axon_hooks")
    _state = {"hook": _hook}
    mod.set_axon_ntff_profile_hook = lambda h: _state.__setitem__("hook", h)
    mod.get_axon_ntff_profile_hook = lambda: _state["hook"]
    sys.modules["antenv.axon_hooks"] = mod

    # zero-egress container: don't try to upload artifacts
    import concourse.bass_utils as bu
    bu.upload_artifacts = lambda tmpdir: f"local:{tmpdir}"


# ----------------------------------------------------------------------
# entry point
# ----------------------------------------------------------------------

def kernel(r_ij, d_ij, species, atom_index12, central_atom_index,
           pair_index12, sign12):
    r_ij = np.asarray(r_ij)
    in_dtype = r_ij.dtype
    n = np.asarray(species).shape[0]
    shapes, in_maps, (row_of_atom, colmap) = preprocess(
        r_ij, d_ij, species, atom_index12, central_atom_index,
        pair_index12, sign12)
    nb_core = len(shapes)

    nc = build_kernel(shapes, nb_core)
    nc.finalize()
    _strip_dead_reg_insts(nc)

    from concourse.bass_utils import run_bass_kernel_spmd
    trace = bool(os.environ.get("ANI_TRACE"))
    if trace:
        _install_trace_hook()
    res = run_bass_kernel_spmd(nc, in_maps, core_ids=list(range(NCORES)),
                               trace=trace)
    if trace and res.exec_time_ns is not None:
        print(f"HW exec time: {res.exec_time_ns} ns")
        kernel.last_exec_time_ns = res.exec_time_ns
        kernel.last_results = res
    dev = np.concatenate([res.results[c]["out"] for c in range(NCORES)], 0)
    rows = dev[row_of_atom]
    out = np.take_along_axis(rows, colmap, axis=1)
    return out.astype(in_dtype, copy=False)


kernel.last_exec_time_ns = None
